# revision 12
# baseline (speedup 1.0000x reference)
"""Trainium2 Bass kernel for CustomSNNLoss (nn_CustomSNNLoss_36429912604816).

Strategy (data-parallel over rows of the NxN similarity):
  - Host: normalize x (O(N*D), trivial), pre-transpose to xnT [D=128, N],
    build per-key-tile one-hot label matrices.
  - Each of the 8 cores owns R = N/8 = 768 query rows. Per core, for each
    key-tile k (48 tiles of 128 keys) the device computes, entirely in SBUF:
        simT[key, q] = xnT[:, ktile].T @ xnq          (f32r matmul, PSUM)
        Sb = exp(2 * sim)   (ScalarE)                  [keys on partitions]
        St = Sb * Sb        (VectorE; == exp(4*sim) when t == 0.25)
    and accumulates class-segment sums via PE matmuls with one-hot weights:
        acc_t[c, q]  += onehot_target[ktile].T @ St    (20 x 384, PSUM acc)
        acc_b[cc, q] += onehot_combo[ktile].T  @ Sb    (100 x 384, PSUM acc)
    The NxN matrix never touches HBM.
  - Host epilogue (O(N) work): per-row pos/neg sums from the class sums,
    -log losses, validity masks, class-weighted means, final scalar.
"""

import os
import numpy as np

N, D = 6144, 128
P = 128                 # partitions / contraction tile
NCORES = 8
R = N // NCORES         # 768 query rows per core
KT = N // P             # 48 key tiles
QC = 2                  # query chunks per core
QF = R // QC            # 384 free-dim per matmul
NT, NB = 20, 5          # target classes, batch keys
NCB = NT * NB           # 100 combined classes
MT = NT                 # acc_t rows (target one-hot)
MB = NCB                # acc_b rows (combo one-hot)
OUT_ROWS = MT + MB      # 120
MIN_T, MAX_T = 0.1, 1.0
TEMP_BATCH = 0.5
EPS = 1e-8

_compile_cache = {}
LAST_RESULT = None  # BassKernelResults from the most recent device run


def _build(scale_t: float, scale_b: float, square_mode: bool):
    from contextlib import ExitStack

    import concourse.bacc as bacc
    import concourse.mybir as mybir
    import concourse.tile as tile

    f32 = mybir.dt.float32
    f32r = mybir.dt.float32r
    EXP = mybir.ActivationFunctionType.Exp

    nc = bacc.Bacc("TRN2", target_bir_lowering=False, debug=False,
                   enable_asserts=False)

    xnt = nc.dram_tensor("xnt", [KT, P, P], f32r, kind="ExternalInput").ap()
    xnq = nc.dram_tensor("xnq", [P, R], f32r, kind="ExternalInput").ap()
    wt = nc.dram_tensor("wt", [P, KT * MT], f32r, kind="ExternalInput").ap()
    wb = nc.dram_tensor("wb", [P, KT * MB], f32r, kind="ExternalInput").ap()
    out_t = nc.dram_tensor("out_t", [MT, R], f32, kind="ExternalOutput").ap()
    out_b = nc.dram_tensor("out_b", [MB, R], f32, kind="ExternalOutput").ap()

    with tile.TileContext(nc) as tc, ExitStack() as ctx:
        const = ctx.enter_context(tc.tile_pool(name="const", bufs=1))
        work = ctx.enter_context(tc.tile_pool(name="work", bufs=6))
        psim = ctx.enter_context(tc.tile_pool(name="psim", bufs=4, space="PSUM"))
        pacc = ctx.enter_context(tc.tile_pool(name="pacc", bufs=1, space="PSUM"))

        xnT_sb = const.tile([P, N], f32r, name="xnT_sb")
        xnq_sb = const.tile([P, R], f32r, name="xnq_sb")
        wt_sb = const.tile([P, KT * MT], f32r, name="wt_sb")
        wb_sb = const.tile([P, KT * MB], f32r, name="wb_sb")

        # Issue order matters: the k-loop needs xnq + xnt[k] + wt/wb chunk 0
        # early; interleave so no consumer waits on a late-queued transfer.
        def load_xnt(k):
            nc.sync.dma_start(xnT_sb[:, k * P:(k + 1) * P], xnt[k, :, :])

        def load_wt(c, n=2):
            sl = slice(c * (KT * MT) // n, (c + 1) * (KT * MT) // n)
            nc.sync.dma_start(wt_sb[:, sl], wt[:, sl])

        def load_wb(c, n=8):
            sl = slice(c * (KT * MB) // n, (c + 1) * (KT * MB) // n)
            nc.sync.dma_start(wb_sb[:, sl], wb[:, sl])

        nc.sync.dma_start(xnq_sb[:, 0:QF], xnq[:, 0:QF])
        nc.sync.dma_start(xnq_sb[:, QF:R], xnq[:, QF:R])
        load_xnt(0)
        load_xnt(1)
        load_wt(0)
        load_wb(0)
        load_wb(1)
        for k in range(2, 6):
            load_xnt(k)
        load_wt(1)
        load_wb(2)
        load_wb(3)
        for k in range(6, 12):
            load_xnt(k)
        for c in range(4, 8):
            load_wb(c)
        for k in range(12, KT):
            load_xnt(k)

        acc_t = [pacc.tile([MT, QF], f32, tag=f"acct{q}", name=f"acct{q}")
                 for q in range(QC)]
        acc_b = [pacc.tile([MB, QF], f32, tag=f"accb{q}", name=f"accb{q}")
                 for q in range(QC)]

        for k in range(KT):
            ksl = slice(k * P, (k + 1) * P)
            for q in range(QC):
                qsl = slice(q * QF, (q + 1) * QF)
                sim = psim.tile([P, QF], f32, tag="sim", name="sim")
                nc.tensor.matmul(
                    sim[:],
                    xnT_sb[:, ksl],
                    xnq_sb[:, qsl],
                    start=True,
                    stop=True,
                )
                sb = work.tile([P, QF], f32r, tag="sb", name="sb")
                nc.scalar.activation(sb[:], sim[:], EXP, scale=scale_b)
                st = work.tile([P, QF], f32r, tag="st", name="st")
                if square_mode:
                    nc.vector.tensor_mul(st[:], sb[:], sb[:])
                else:
                    nc.scalar.activation(st[:], sim[:], EXP, scale=scale_t)
                nc.tensor.matmul(
                    acc_b[q][:],
                    wb_sb[:, k * MB:(k + 1) * MB],
                    sb[:],
                    start=(k == 0),
                    stop=(k == KT - 1),
                )
                nc.tensor.matmul(
                    acc_t[q][:],
                    wt_sb[:, k * MT:(k + 1) * MT],
                    st[:],
                    start=(k == 0),
                    stop=(k == KT - 1),
                )

        out_t_sb = const.tile([MT, R], f32, name="out_t_sb")
        out_b_sb = const.tile([MB, R], f32, name="out_b_sb")
        for q in range(QC):
            qsl = slice(q * QF, (q + 1) * QF)
            nc.vector.tensor_copy(out_t_sb[:, qsl], acc_t[q][:])
            nc.scalar.copy(out_b_sb[:, qsl], acc_b[q][:])
            nc.sync.dma_start(out_t[:, qsl], out_t_sb[:, qsl])
            nc.sync.dma_start(out_b[:, qsl], out_b_sb[:, qsl])

    nc.compile()
    return nc


def _get_compiled(scale_t: float, scale_b: float, square_mode: bool):
    key = (round(scale_t, 9), round(scale_b, 9), square_mode)
    if key not in _compile_cache:
        _compile_cache[key] = _build(scale_t, scale_b, square_mode)
    return _compile_cache[key]


def _round_f32r(v):
    """Round fp32 mantissa to 11 explicit bits (the PE's FP32r format)."""
    b = np.ascontiguousarray(v, dtype=np.float32).view(np.uint32).astype(np.uint64)
    r = ((b + np.uint64(1 << 11)) >> np.uint64(12)) << np.uint64(12)
    return r.astype(np.uint32).view(np.float32)


def _host_prep(input, temperature, targets, batch0):
    x = np.asarray(input, dtype=np.float32)
    t = float(np.clip(np.float32(temperature), MIN_T, MAX_T))
    scale_t = 1.0 / t
    scale_b = 1.0 / TEMP_BATCH
    square_mode = abs(scale_t - 2.0 * scale_b) < 1e-6

    norms = np.sqrt((x * x).sum(axis=1, keepdims=True, dtype=np.float32))
    norms = np.maximum(norms, np.float32(EPS)).astype(np.float32)
    xn = _round_f32r((x / norms).astype(np.float32))       # FP32r operand
    # [KT, P(d), P(j)]: per-key-tile transposed blocks, contiguous in DRAM
    xnt = np.ascontiguousarray(xn.reshape(KT, P, D).transpose(0, 2, 1))
    xnq_all = np.ascontiguousarray(xn.T)                   # [128, 6144]
    s_ii = (xn * xn).sum(axis=1, dtype=np.float32)         # ~1.0, matmul diag

    tg = np.asarray(targets).astype(np.int64)
    bt = np.asarray(batch0).astype(np.int64)
    combo = tg * NB + bt

    kk = np.repeat(np.arange(KT), P).reshape(KT, P)
    pp = np.tile(np.arange(P), KT).reshape(KT, P)
    wt3 = np.zeros((KT, P, MT), dtype=np.float32)
    wt3[kk, pp, tg.reshape(KT, P)] = 1.0
    wb3 = np.zeros((KT, P, MB), dtype=np.float32)
    wb3[kk, pp, combo.reshape(KT, P)] = 1.0
    wt_in = np.ascontiguousarray(wt3.transpose(1, 0, 2).reshape(P, KT * MT))
    wb_in = np.ascontiguousarray(wb3.transpose(1, 0, 2).reshape(P, KT * MB))

    return xnt, xnq_all, s_ii, tg, bt, combo, wt_in, wb_in, scale_t, scale_b, square_mode


def _epilogue(acc, s_ii, tg, bt, combo, weight_target, weight_batch0,
              scale_t, scale_b):
    """acc: [120, N] device class sums. Everything here is O(N)."""
    f = np.float64
    idx = np.arange(N)
    classsum_t = acc[0:MT].astype(f)          # [20, N]
    combosum_b = acc[MT:OUT_ROWS].astype(f)   # [100, N]
    rowsum_t = classsum_t.sum(axis=0)
    classsum_b = combosum_b.reshape(NT, NB, N).sum(axis=1)  # [20, N]

    diag_t = np.exp(scale_t * s_ii.astype(f))
    diag_b = np.exp(scale_b * s_ii.astype(f))

    cnt_t = np.bincount(tg, minlength=NT)
    n_tb = np.zeros((NT, NB), dtype=np.int64)
    np.add.at(n_tb, (tg, bt), 1)

    # ---- target SNN loss ----
    own_t = classsum_t[tg, idx]
    pos_t = own_t - diag_t
    neg_t = rowsum_t - own_t
    cnt_pos = cnt_t[tg]
    cnt_neg = N - cnt_pos
    valid_t = (cnt_pos >= 2) & (cnt_neg >= 1)
    pos_s = np.where(valid_t, pos_t, 1.0)
    neg_s = np.where(valid_t, neg_t, 1.0)
    loss_i = -np.log(pos_s / (pos_s + neg_s))
    lsum = np.bincount(tg, weights=np.where(valid_t, loss_i, 0.0), minlength=NT)
    vcnt = np.bincount(tg, weights=valid_t.astype(f), minlength=NT)
    mean = lsum / np.maximum(vcnt, 1.0)
    wt_w = np.asarray(weight_target).astype(f)
    loss_target = np.where(vcnt > 0, mean * wt_w, 0.0).sum()

    # ---- batch-effect loss ----
    own_tb = combosum_b[combo, idx]
    samet = classsum_b[tg, idx]
    pos_b = own_tb - diag_b
    neg_b = samet - own_tb
    cnt_pos_b = n_tb[tg, bt]
    cnt_neg_b = cnt_t[tg] - cnt_pos_b
    valid_b = (cnt_pos_b >= 2) & (cnt_neg_b >= 1)
    pos_bs = np.where(valid_b, pos_b, 1.0)
    neg_bs = np.where(valid_b, neg_b, 1.0)
    loss_bi = -np.log(pos_bs / (pos_bs + neg_bs))
    inv = np.where(valid_b, 1.0 / np.where(valid_b, loss_bi, 1.0), 0.0)
    lsum_b = np.bincount(bt, weights=inv, minlength=NB)
    vcnt_b = np.bincount(bt, weights=valid_b.astype(f), minlength=NB)
    mean_b = lsum_b / np.maximum(vcnt_b, 1.0)
    wb_w = np.asarray(weight_batch0).astype(f)
    loss_batch = np.where(vcnt_b > 0, mean_b * wb_w, 0.0).sum()

    return np.float32(0.9 * loss_target + 0.1 * loss_batch)


def _run_with_retry(nc, in_maps, core_ids, attempts=3):
    import time as _time

    from concourse.bass_utils import run_bass_kernel_spmd

    for i in range(attempts):
        try:
            return run_bass_kernel_spmd(nc, in_maps, core_ids)
        except Exception:
            if i == attempts - 1:
                raise
            _time.sleep(90)  # transient NRT device errors clear after a pause


def kernel(input, temperature, weight_target, weight_batch0, targets, batch0):
    global LAST_RESULT

    (xnt, xnq_all, s_ii, tg, bt, combo, wt_in, wb_in,
     scale_t, scale_b, square_mode) = _host_prep(input, temperature,
                                                 targets, batch0)

    nc = _get_compiled(scale_t, scale_b, square_mode)

    in_maps = [
        {
            "xnt": xnt,
            "xnq": np.ascontiguousarray(xnq_all[:, c * R:(c + 1) * R]),
            "wt": wt_in,
            "wb": wb_in,
        }
        for c in range(NCORES)
    ]
    LAST_RESULT = _run_with_retry(nc, in_maps, list(range(NCORES)))
    acc = np.concatenate(
        [
            np.concatenate(
                [LAST_RESULT.results[c]["out_t"], LAST_RESULT.results[c]["out_b"]],
                axis=0,
            )
            for c in range(NCORES)
        ],
        axis=1,
    )  # [120, N]

    return _epilogue(acc, s_ii, tg, bt, combo, weight_target, weight_batch0,
                     scale_t, scale_b)
